# revision 15
# baseline (speedup 1.0000x reference)
"""Causal multi-head self-attention (B=2, S=2048, D=768, H=12) on 8 TRN2 NeuronCores.

Sharding: core c = (batch b=c//4, head-group hg=c%4 of 3 heads).
Each core computes Q/K/V for its 3 heads, causal attention, and the partial
output projection sum_h out_h @ Wo[:, h]^T -> (S, D) in fp16. Host sums the
4 head-group partials per batch (the unshard step).

On-core dataflow (transposed (feature, seq) layout, f32r matmuls), arranged
as one globally-woven instruction stream that keeps TensorE dense (the HAM
clock-gate re-throttles the PE to 1.2 GHz after any ~3.4us window with idle
gaps, which doubles every matmul):

  region 0: QKV^T chains for s-chunks 0,1 (psum[m,s] += WcatT[i,m].T @ XT[i,s]
     per 512-col chunk, 8-bank rotation) + V' transposes. DMA descriptors for
     X/weights are generated on sync+scalar+gpsimd in parallel, first chunk
     prioritized.
  region 1: attention q-half 0 for all 3 heads (24 pipelined steps), woven
     with the QKV chains + V' transposes of s-chunks 2,3 (the PE filler that
     keeps it busy while ACT exps).
  region 2: attention q-half 1 (48 steps), woven with output projection
     blocks for q-chunks 0,1 as their divides complete; projection of
     q-chunks 2,3 trails at the end.

Attention step pipeline (scores run R=2 steps ahead of PV):
  scoresT[k,q] = KT.T @ QT per 1024-wide q-half -> exp on ACT -> fp16 expt
  (NO pre-exp mask: the diagonal window is zeroed AFTER exp by a DVE
  multiply with a 0/1 fp16 mask, so ACT never waits on DVE) ->
  PV: pout[qc] += V'[t].T @ expT (65 rows: 64 data + denominator).
  Per-(h,qc) epilogue: recip(den) -> broadcast -> numerator * recip.

PSUM: scores (128,1024)=2 banks x2 + 2 pout banks + 2 filler banks = 8.
"""

import numpy as np
from contextlib import ExitStack

import concourse.bass as bass
import concourse.tile as tile
from concourse import bacc, mybir
from concourse import bass_utils

F32 = mybir.dt.float32
F32R = mybir.dt.float32r
BF16 = mybir.dt.bfloat16
FP16 = mybir.dt.float16
AF = mybir.ActivationFunctionType

B, S, D, H = 2, 2048, 768, 12
DK = 64
HPC = 3            # heads per core
NCORES = 8
NI = D // 128      # 6 input-feature chunks
NM = 5             # output m-chunks of 128 (640 rows incl. 64 pad)
NT = S // 128      # 16 k-tiles
NQC = S // 512     # 4 q-chunks

# per-local-head (base_partition, m_chunk) in the QKVT buffer
QPOS = [(0, 0), (64, 0), (0, 2)]
KPOS = [(0, 1), (64, 1), (0, 3)]
VPOS = [(64, 2), (64, 3), (0, 4)]

_NC_CACHE = {}


def build_nc(dbg=False):
    key = ("nc", dbg)
    if key in _NC_CACHE:
        return _NC_CACHE[key]
    nc = bacc.Bacc("TRN2", target_bir_lowering=False, debug=False,
                   num_devices=NCORES)

    xt_d = nc.dram_tensor("xt", [NI, 128, S], F32R, kind="ExternalInput").ap()
    wcat_d = nc.dram_tensor("wcat", [NI, 128, NM * 128], F32R, kind="ExternalInput").ap()
    wot_d = nc.dram_tensor("wot", [2, 128, D], FP16, kind="ExternalInput").ap()
    mask_d = nc.dram_tensor("mask", [128, 128], FP16, kind="ExternalInput").ap()
    id_d = nc.dram_tensor("ident", [128, 128], F32R, kind="ExternalInput").ap()
    out_d = nc.dram_tensor("out", [S, D], FP16, kind="ExternalOutput").ap()

    with tile.TileContext(nc) as tc, ExitStack() as ctx:
        const = ctx.enter_context(tc.tile_pool(name="const", bufs=1))

        # persistent SBUF buffers
        xt = const.tile([128, NI, S], F32R)             # X^T
        wcat = const.tile([128, NI, NM * 128], F32R)    # W^T (QKV packed)
        wot = const.tile([128, 2, D], FP16)             # Wo^T [h0;h1],[h2;pad]
        mask01 = const.tile([128, 128], FP16)           # 0/1 causal window mask
        ident = const.tile([128, 128], F32R)
        qkvt = const.tile([128, NM, S], F32R)           # Q^T/K^T/V^T packed
        vp = const.tile([128, HPC, NT, DK + 1], FP16)   # V' = [V | ones]
        oct_ = const.tile([128, 2, S], FP16)            # packed out^T [h0;h1],[h2]
        qk16 = const.tile([128, 4, S], FP16)            # fp16 Q/K for attention
        junk = const.tile([128, 512], FP16)             # PE warmup fodder

        # DMA issue order = arrival priority. Spread descriptor generation
        # over three otherwise-idle engine queues; region 0's critical path
        # (wcat m-chunk-0 columns + the first s-chunk of X) goes first.
        ENGS = (nc.sync, nc.scalar, nc.gpsimd)
        for i in range(NI):
            ENGS[i % 3].dma_start(wcat[:, i, 0:128], wcat_d[i][:, 0:128])
        for i in range(NI):
            ENGS[i % 3].dma_start(xt[:, i, 0:512], xt_d[i][:, 0:512])
        for i in range(NI):
            ENGS[i % 3].dma_start(wcat[:, i, 128:NM * 128],
                                  wcat_d[i][:, 128:NM * 128])
        nc.sync.dma_start(ident[:], id_d)
        nc.gpsimd.dma_start(mask01[:], mask_d)
        nc.sync.dma_start(wot[:], wot_d.rearrange("c p f -> p c f"))
        nc.vector.memset(vp[:, :, :, DK:DK + 1], 1.0)   # denominator ones col
        nc.vector.memset(junk[:], 1.0)
        for sc in range(1, NQC):
            for i in range(NI):
                ENGS[(sc * NI + i) % 3].dma_start(
                    xt[:, i, sc * 512:(sc + 1) * 512],
                    xt_d[i][:, sc * 512:(sc + 1) * 512])

        # ---- QKV^T projection chain + V' transpose emitters (shared by
        # region 0 and the weave)
        def qkv_chain(pool, tag, sc, m):
            s0 = sc * 512
            pq = pool.tile([128, 512], F32, tag=tag, name=f"pq{sc}_{m}")
            for i in range(NI):
                nc.tensor.matmul(
                    pq[:],
                    wcat[:, i, m * 128:(m + 1) * 128],
                    xt[:, i, s0:s0 + 512],
                    start=(i == 0), stop=(i == NI - 1),
                )
            nc.vector.tensor_copy(qkvt[:, m, s0:s0 + 512], pq[:])
            # fp16 shadow of Q/K rows for the attention core
            if m <= 1:
                nc.vector.tensor_copy(qk16[:, m, s0:s0 + 512], pq[:])
            elif m <= 3:
                nc.vector.tensor_copy(qk16[0:DK, m, s0:s0 + 512], pq[0:DK, :])

        def v_transposes(pool, tag, sc, h):
            vb, vchunk = VPOS[h]
            for t in range(4 * sc, 4 * sc + 4):
                ptr = pool.tile([128, DK], F32R, tag=tag, name=f"tr{h}_{t}")
                nc.tensor.transpose(
                    ptr[:],
                    qkvt[vb:vb + DK, vchunk, t * 128:(t + 1) * 128],
                    ident[vb:vb + DK, vb:vb + DK],
                )
                nc.vector.tensor_copy(vp[:, h, t, 0:DK], ptr[:])

        # ---- region 0: s-chunks 0,1 on a deep 7-bank rotation, preceded by
        # a warmup burst of junk matmuls during the DMA lead-in so the HAM
        # clock-gate reaches 2.4 GHz before the first real chain issues
        with tc.tile_pool(name="ps_ab", bufs=7, space="PSUM") as ps_ab:
            wtile = ps_ab.tile([128, 512], F32, tag="warm", bufs=1,
                               name="warm")
            for w in range(26):
                nc.tensor.matmul(wtile[:], junk[:, 0:128], junk[:],
                                 start=True, stop=True)
            for sc in (0, 1):
                for m in range(NM):
                    qkv_chain(ps_ab, "ab", sc, m)
                for h in range(HPC):
                    v_transposes(ps_ab, "ab", sc, h)

        # ---- regions 1+2: woven attention + trailing QKV + projection
        with tc.tile_pool(name="ps_s", bufs=2, space="PSUM") as ps_s, \
             tc.tile_pool(name="ps_o", bufs=2, space="PSUM") as ps_o, \
             tc.tile_pool(name="ps_f", bufs=2, space="PSUM") as ps_f, \
             tc.tile_pool(name="sb_exp", bufs=6) as sb_exp, \
             tc.tile_pool(name="sb_div", bufs=3) as sb_div:

            pouts = {}

            def score_step(h, qp, t):
                qb, qchunk = QPOS[h]
                kb, kchunk = KPOS[h]
                qcs = (2 * qp, 2 * qp + 1)
                qc_lo = t // 4
                off = 128 * (t % 4)   # diag col offset inside qc_lo's half
                pscr = ps_s.tile([128, 1024], F32, tag="scr",
                                 name=f"sc{h}_{qp}_{t}")
                for half, qc in enumerate(qcs):
                    if qc < qc_lo:
                        continue
                    cs = off if qc == qc_lo else 0  # skip fully-masked cols
                    nc.tensor.matmul(
                        pscr[:, half * 512 + cs:(half + 1) * 512],
                        qk16[kb:kb + DK, kchunk, t * 128:(t + 1) * 128],
                        qk16[qb:qb + DK, qchunk,
                             qc * 512 + cs:(qc + 1) * 512],
                        start=True, stop=True,
                    )
                lo = (512 if qc_lo == qcs[1] else 0) + \
                     (off if qc_lo in qcs else 0)
                expt = sb_exp.tile([128, 1024], FP16, tag="exp",
                                   name=f"ex{h}_{qp}_{t}")
                nc.scalar.activation(expt[:, lo:1024], pscr[:, lo:1024],
                                     AF.Exp)
                if qc_lo in qcs:
                    # zero the above-diagonal weights AFTER exp (0/1 fp16
                    # mask on DVE) so ACT never waits on another engine
                    w = (qc_lo - 2 * qp) * 512 + off
                    nc.vector.tensor_mul(expt[:, w:w + 128],
                                         expt[:, w:w + 128], mask01[:])
                return expt

            def pv_step(h, qp, t, expt):
                qcs = (2 * qp, 2 * qp + 1)
                qc_lo = t // 4
                off = 128 * (t % 4)
                for half, qc in enumerate(qcs):
                    if qc < qc_lo:
                        continue
                    cs = off if qc == qc_lo else 0
                    nc.tensor.matmul(
                        pouts[(h, qc)][:, cs:512],
                        vp[:, h, t, :],
                        expt[:, half * 512 + cs:(half + 1) * 512],
                        start=(t == 0), stop=(t == 4 * qc + 3),
                    )

            def divide(h, qc):
                # evict the finished chain at once so its PSUM bank frees
                # immediately; the slow recip/divide runs off the copy
                pout = pouts.pop((h, qc))
                nout = sb_div.tile([DK + 1, 512], F32, tag="nout",
                                   name=f"no{h}_{qc}")
                nc.vector.tensor_copy(nout[:], pout[:])
                # reciprocal of the denominator row, then broadcast it over
                # 64 partitions with a rank-1 matmul (ones64^T @ rc) into the
                # pout slot just freed — far lower latency than the old
                # DMA-spread + gpsimd-broadcast chain
                rc = sb_div.tile([1, 512], F32, tag="rc",
                                 name=f"rc{h}_{qc}")
                nc.vector.reciprocal(rc[:], nout[DK:DK + 1, :])
                prb = sb_div.tile([DK, 512], F32, tag="prb",
                                  name=f"prb{h}_{qc}")
                nc.gpsimd.partition_broadcast(prb[:], rc[:])
                if h == 1:
                    # h1 lands at partitions 64-127: shift via SBUF DMA
                    tmp = sb_div.tile([DK, 512], FP16, tag="tmp",
                                      name=f"tmp{h}_{qc}")
                    nc.vector.tensor_mul(tmp[:], nout[0:DK, :], prb[:])
                    nc.sync.dma_start(
                        oct_[DK:128, 0, qc * 512:(qc + 1) * 512], tmp[:])
                else:
                    nc.vector.tensor_mul(
                        oct_[0:DK, h // 2, qc * 512:(qc + 1) * 512],
                        nout[0:DK, :], prb[:],
                    )

            def d_proj(qt):
                # output projection for one 128-row q-tile, split into two
                # single-bank psum tiles so it can borrow the filler pool
                pa = ps_f.tile([128, 512], F32, tag="fil", name=f"pa{qt}")
                pb = ps_f.tile([128, 256], F32, tag="fil", name=f"pb{qt}")
                for c, kk in ((0, 128), (1, DK)):
                    nc.tensor.matmul(pa[:], oct_[0:kk, c, qt * 128:(qt + 1) * 128],
                                     wot[0:kk, c, 0:512],
                                     start=(c == 0), stop=(c == 1))
                for c, kk in ((0, 128), (1, DK)):
                    nc.tensor.matmul(pb[:], oct_[0:kk, c, qt * 128:(qt + 1) * 128],
                                     wot[0:kk, c, 512:D],
                                     start=(c == 0), stop=(c == 1))
                ot = sb_exp.tile([128, D], FP16, tag="exp", name=f"ot{qt}")
                nc.vector.tensor_copy(ot[:, 0:512], pa[:])
                nc.vector.tensor_copy(ot[:, 512:D], pb[:])
                nc.sync.dma_start(out_d[qt * 128:(qt + 1) * 128, :], ot[:])

            # PE filler work, scheduled at fixed pipeline positions k:
            #   region 1 (k 0..23, attention q-half 0): QKV chains + V'
            #   transposes of s-chunks 2,3 (sc3's Q chunks m0/m2 must land
            #   before k=24 when q-half-1 scores start).
            #   region 2: projection of q-chunks 0,1 once all heads' divides
            #   for them completed (k>=26 / k>=40); q-chunk 2 near the end.
            fills = {}
            for n, m in enumerate(range(NM)):
                fills.setdefault(1 + 2 * n, []).append(
                    lambda m=m: qkv_chain(ps_f, "fil", 2, m))
            for h in range(HPC):
                fills.setdefault(11 + h, []).append(
                    lambda h=h: v_transposes(ps_f, "fil", 2, h))
            # sc3: Q chunks (m0/m2) must land before k=24; V-carrying chunks
            # (m2/m3/m4) before their transposes; only m1 (pure K, unread
            # until k~36) shifts into region 2 where the PE has slack
            for n, m in enumerate((0, 2, 4, 3)):
                fills.setdefault(14 + 2 * n, []).append(
                    lambda m=m: qkv_chain(ps_f, "fil", 3, m))
            for h in range(HPC):
                fills.setdefault(21 + h, []).append(
                    lambda h=h: v_transposes(ps_f, "fil", 3, h))
            for n, m in enumerate((1,)):
                fills.setdefault(26 + 2 * n, []).append(
                    lambda m=m: qkv_chain(ps_f, "fil", 3, m))
            for n, qt in enumerate(range(0, 4)):        # q-chunk 0
                fills.setdefault(30 + 3 * n, []).append(
                    lambda qt=qt: d_proj(qt))
            for n, qt in enumerate(range(4, 8)):        # q-chunk 1
                fills.setdefault(42 + 3 * n, []).append(
                    lambda qt=qt: d_proj(qt))
            for n, qt in enumerate(range(8, 12)):       # q-chunk 2
                fills.setdefault(70 + n, []).append(
                    lambda qt=qt: d_proj(qt))

            # Flat pipelined stream over all (head, q-half, k-tile) steps,
            # q-half-major so region 1 only needs s-chunks 0,1. Scores run
            # R=2 ahead of PV (matching the 2-slot score pool) so the PE
            # always has queued matmuls while ACT computes the exps.
            sc_list = [(h, 0, t) for h in range(HPC) for t in range(8)] + \
                      [(h, 1, t) for h in range(HPC) for t in range(16)]
            R = 2
            expts = {}
            for k in range(len(sc_list) + R):
                if k < len(sc_list):
                    expts[sc_list[k]] = score_step(*sc_list[k])
                j = k - R
                if j >= 0:
                    h, qp, t = sc_list[j]
                    if t == 0:
                        for qc in (2 * qp, 2 * qp + 1):
                            pouts[(h, qc)] = ps_o.tile([DK + 1, 512], F32,
                                                       tag="pout",
                                                       name=f"po{h}_{qc}")
                    pv_step(h, qp, t, expts.pop(sc_list[j]))
                    if t == 4 * (2 * qp) + 3:
                        divide(h, 2 * qp)
                    if t == 4 * (2 * qp + 1) + 3:
                        divide(h, 2 * qp + 1)
                for fn in fills.pop(k, ()):
                    fn()

            for fns in sorted(fills):
                for fn in fills[fns]:
                    fn()
            for qt in range(12, 16):                    # q-chunk 3 trails
                d_proj(qt)

    nc.compile()
    _NC_CACHE[key] = nc
    return nc


def make_in_maps(X, Wq, Wk, Wv, Wo):
    X = np.ascontiguousarray(np.asarray(X, dtype=np.float32))
    Wq = np.asarray(Wq, dtype=np.float32)
    Wk = np.asarray(Wk, dtype=np.float32)
    Wv = np.asarray(Wv, dtype=np.float32)
    Wo = np.asarray(Wo, dtype=np.float32)

    # causal 0/1 window mask: keep q >= k; rows=k (p), cols=q (f)
    p = np.arange(128)[:, None]
    f = np.arange(128)[None, :]
    mask = (f >= p).astype(np.float16)
    ident = np.eye(128, dtype=np.float32)

    in_maps = []
    for c in range(NCORES):
        b, hg = c // 4, c % 4
        gh = [hg * HPC + l for l in range(HPC)]
        q = [Wq[g * DK:(g + 1) * DK, :] / 8.0 for g in gh]
        k = [Wk[g * DK:(g + 1) * DK, :] for g in gh]
        v = [Wv[g * DK:(g + 1) * DK, :] for g in gh]
        wcat_rows = np.vstack([
            q[0], q[1], k[0], k[1], q[2], v[0], k[2], v[1], v[2],
            np.zeros((DK, D), dtype=np.float32),
        ])                                            # (640, 768)
        wcat = np.ascontiguousarray(wcat_rows.T.reshape(NI, 128, NM * 128))
        w0, w1, w2 = (Wo[:, g * DK:(g + 1) * DK].T for g in gh)
        wot = np.ascontiguousarray(np.stack([
            np.vstack([w0, w1]),
            np.vstack([w2, np.zeros((DK, D), dtype=np.float32)]),
        ]).astype(np.float16))                                     # (2, 128, 768)
        xt = np.ascontiguousarray(X[b].T.reshape(NI, 128, S))
        in_maps.append({
            "xt": xt, "wcat": wcat, "wot": wot,
            "mask": mask, "ident": ident,
        })
    return in_maps


def _run(in_maps, trace=False, trace_cores=None):
    nc = build_nc()
    return bass_utils.run_bass_kernel_spmd(
        nc, in_maps, core_ids=list(range(NCORES)),
        trace=trace, trace_cores=trace_cores,
    )


def kernel(X, Wq, Wk, Wv, Wo):
    in_maps = make_in_maps(X, Wq, Wk, Wv, Wo)
    res = _run(in_maps, trace=False)
    out = np.zeros((B, S, D), dtype=np.float32)
    for c in range(NCORES):
        out[c // 4] += res.results[c]["out"]
    return out


# revision 17
# speedup vs baseline: 1.1154x; 1.1154x over previous
"""Causal multi-head self-attention (B=2, S=2048, D=768, H=12) on 8 TRN2 NeuronCores.

Sharding: core c = (batch b=c//4, head-group hg=c%4 of 3 heads).
Each core computes Q/K/V for its 3 heads, causal attention, and the partial
output projection sum_h out_h @ Wo[:, h]^T -> (S, D) in fp16. Host sums the
4 head-group partials per batch (the unshard step).

On-core dataflow (transposed (feature, seq) layout, f32r matmuls), arranged
as one globally-woven instruction stream that keeps TensorE dense (the HAM
clock-gate re-throttles the PE to 1.2 GHz after any ~3.4us window with idle
gaps, which doubles every matmul):

  region 0: QKV^T chains for s-chunks 0,1 (psum[m,s] += WcatT[i,m].T @ XT[i,s]
     per 512-col chunk, 8-bank rotation) + V' transposes. DMA descriptors for
     X/weights are generated on sync+scalar+gpsimd in parallel, first chunk
     prioritized.
  region 1: attention q-half 0 for all 3 heads (24 pipelined steps), woven
     with the QKV chains + V' transposes of s-chunks 2,3 (the PE filler that
     keeps it busy while ACT exps).
  region 2: attention q-half 1 (48 steps), woven with output projection
     blocks for q-chunks 0,1 as their divides complete; projection of
     q-chunks 2,3 trails at the end.

Attention step pipeline (scores run R=2 steps ahead of PV):
  scoresT[k,q] = KT.T @ QT per 1024-wide q-half -> exp on ACT -> fp16 expt
  (NO pre-exp mask: the diagonal window is zeroed AFTER exp by a DVE
  multiply with a 0/1 fp16 mask, so ACT never waits on DVE) ->
  PV: pout[qc] += V'[t].T @ expT (65 rows: 64 data + denominator).
  Per-(h,qc) epilogue: recip(den) -> broadcast -> numerator * recip.

PSUM: scores (128,1024)=2 banks x2 + 2 pout banks + 2 filler banks = 8.
"""

import numpy as np
from contextlib import ExitStack

import concourse.bass as bass
import concourse.tile as tile
from concourse import bacc, mybir
from concourse import bass_utils

F32 = mybir.dt.float32
F32R = mybir.dt.float32r
BF16 = mybir.dt.bfloat16
FP16 = mybir.dt.float16
AF = mybir.ActivationFunctionType

B, S, D, H = 2, 2048, 768, 12
DK = 64
HPC = 3            # heads per core
NCORES = 8
NI = D // 128      # 6 input-feature chunks
NM = 5             # output m-chunks of 128 (640 rows incl. 64 pad)
NT = S // 128      # 16 k-tiles
NQC = S // 512     # 4 q-chunks

# per-local-head (base_partition, m_chunk) in the QKVT buffer
QPOS = [(0, 0), (64, 0), (0, 2)]
KPOS = [(0, 1), (64, 1), (0, 3)]
VPOS = [(64, 2), (64, 3), (0, 4)]

_NC_CACHE = {}


def build_nc(dbg=False):
    key = ("nc", dbg)
    if key in _NC_CACHE:
        return _NC_CACHE[key]
    nc = bacc.Bacc("TRN2", target_bir_lowering=False, debug=False,
                   num_devices=NCORES)

    xt_d = nc.dram_tensor("xt", [NI, 128, S], F32R, kind="ExternalInput").ap()
    wcat_d = nc.dram_tensor("wcat", [NI, 128, NM * 128], F32R, kind="ExternalInput").ap()
    wot_d = nc.dram_tensor("wot", [2, 128, D], FP16, kind="ExternalInput").ap()
    mask_d = nc.dram_tensor("mask", [128, 128], FP16, kind="ExternalInput").ap()
    id_d = nc.dram_tensor("ident", [128, 128], F32R, kind="ExternalInput").ap()
    out_d = nc.dram_tensor("out", [S, D], FP16, kind="ExternalOutput").ap()

    with tile.TileContext(nc) as tc, ExitStack() as ctx:
        const = ctx.enter_context(tc.tile_pool(name="const", bufs=1))

        # persistent SBUF buffers
        xt = const.tile([128, NI, S], F32R)             # X^T
        wcat = const.tile([128, NI, NM * 128], F32R)    # W^T (QKV packed)
        wot = const.tile([128, 2, D], FP16)             # Wo^T [h0;h1],[h2;pad]
        mask01 = const.tile([128, 128], FP16)           # 0/1 causal window mask
        ident = const.tile([128, 128], F32R)
        qkvt = const.tile([128, NM, S], F32R)           # Q^T/K^T/V^T packed
        vp = const.tile([128, HPC, NT, DK + 1], FP16)   # V' = [V | ones]
        oct_ = const.tile([128, 2, S], FP16)            # packed out^T [h0;h1],[h2]
        qk16 = const.tile([128, 4, S], FP16)            # fp16 Q/K for attention
        junk = const.tile([128, 512], FP16)             # PE warmup fodder

        # DMA issue order = arrival priority. Spread descriptor generation
        # over three otherwise-idle engine queues; region 0's critical path
        # (wcat m-chunk-0 columns + the first s-chunk of X) goes first.
        ENGS = (nc.sync, nc.scalar, nc.gpsimd)
        for i in range(NI):
            ENGS[i % 3].dma_start(wcat[:, i, 0:128], wcat_d[i][:, 0:128])
        for i in range(NI):
            ENGS[i % 3].dma_start(xt[:, i, 0:512], xt_d[i][:, 0:512])
        for i in range(NI):
            ENGS[i % 3].dma_start(wcat[:, i, 128:NM * 128],
                                  wcat_d[i][:, 128:NM * 128])
        nc.sync.dma_start(ident[:], id_d)
        nc.vector.memset(vp[:, :, :, DK:DK + 1], 1.0)   # denominator ones col
        nc.vector.memset(junk[:], 1.0)
        for i in range(NI):
            ENGS[i % 3].dma_start(xt[:, i, 512:1024], xt_d[i][:, 512:1024])
        nc.gpsimd.dma_start(mask01[:], mask_d)
        for sc in range(2, NQC):
            for i in range(NI):
                ENGS[(sc * NI + i) % 3].dma_start(
                    xt[:, i, sc * 512:(sc + 1) * 512],
                    xt_d[i][:, sc * 512:(sc + 1) * 512])
        nc.sync.dma_start(wot[:], wot_d.rearrange("c p f -> p c f"))

        # ---- QKV^T projection chain + V' transpose emitters (shared by
        # region 0 and the weave)
        def qkv_chain(pool, tag, sc, m):
            s0 = sc * 512
            pq = pool.tile([128, 512], F32, tag=tag, name=f"pq{sc}_{m}")
            for i in range(NI):
                nc.tensor.matmul(
                    pq[:],
                    wcat[:, i, m * 128:(m + 1) * 128],
                    xt[:, i, s0:s0 + 512],
                    start=(i == 0), stop=(i == NI - 1),
                )
            nc.vector.tensor_copy(qkvt[:, m, s0:s0 + 512], pq[:])
            # fp16 shadow of Q/K rows for the attention core
            if m <= 1:
                nc.vector.tensor_copy(qk16[:, m, s0:s0 + 512], pq[:])
            elif m <= 3:
                nc.vector.tensor_copy(qk16[0:DK, m, s0:s0 + 512], pq[0:DK, :])

        def v_transposes(pool, tag, sc, h):
            vb, vchunk = VPOS[h]
            for t in range(4 * sc, 4 * sc + 4):
                ptr = pool.tile([128, DK], F32R, tag=tag, name=f"tr{h}_{t}")
                nc.tensor.transpose(
                    ptr[:],
                    qkvt[vb:vb + DK, vchunk, t * 128:(t + 1) * 128],
                    ident[vb:vb + DK, vb:vb + DK],
                )
                nc.vector.tensor_copy(vp[:, h, t, 0:DK], ptr[:])

        # ---- region 0: s-chunks 0,1 on a deep 7-bank rotation, preceded by
        # a warmup burst of junk matmuls during the DMA lead-in so the HAM
        # clock-gate reaches 2.4 GHz before the first real chain issues
        with tc.tile_pool(name="ps_ab", bufs=7, space="PSUM") as ps_ab:
            wtile = ps_ab.tile([128, 512], F32, tag="warm", bufs=1,
                               name="warm")
            for w in range(26):
                nc.tensor.matmul(wtile[:], junk[:, 0:128], junk[:],
                                 start=True, stop=True)
            for sc in (0, 1):
                for m in range(NM):
                    qkv_chain(ps_ab, "ab", sc, m)
                for h in range(HPC):
                    v_transposes(ps_ab, "ab", sc, h)

        # ---- regions 1+2: woven attention + trailing QKV + projection
        with tc.tile_pool(name="ps_s", bufs=2, space="PSUM") as ps_s, \
             tc.tile_pool(name="ps_o", bufs=2, space="PSUM") as ps_o, \
             tc.tile_pool(name="ps_f", bufs=2, space="PSUM") as ps_f, \
             tc.tile_pool(name="sb_exp", bufs=6) as sb_exp, \
             tc.tile_pool(name="sb_div", bufs=3) as sb_div:

            pouts = {}

            def score_step(h, qp, t):
                qb, qchunk = QPOS[h]
                kb, kchunk = KPOS[h]
                qcs = (2 * qp, 2 * qp + 1)
                qc_lo = t // 4
                off = 128 * (t % 4)   # diag col offset inside qc_lo's half
                pscr = ps_s.tile([128, 1024], F32, tag="scr",
                                 name=f"sc{h}_{qp}_{t}")
                for half, qc in enumerate(qcs):
                    if qc < qc_lo:
                        continue
                    cs = off if qc == qc_lo else 0  # skip fully-masked cols
                    nc.tensor.matmul(
                        pscr[:, half * 512 + cs:(half + 1) * 512],
                        qk16[kb:kb + DK, kchunk, t * 128:(t + 1) * 128],
                        qk16[qb:qb + DK, qchunk,
                             qc * 512 + cs:(qc + 1) * 512],
                        start=True, stop=True,
                    )
                lo = (512 if qc_lo == qcs[1] else 0) + \
                     (off if qc_lo in qcs else 0)
                expt = sb_exp.tile([128, 1024], FP16, tag="exp",
                                   name=f"ex{h}_{qp}_{t}")
                nc.scalar.activation(expt[:, lo:1024], pscr[:, lo:1024],
                                     AF.Exp)
                if qc_lo in qcs:
                    # zero the above-diagonal weights AFTER exp (0/1 fp16
                    # mask on DVE) so ACT never waits on another engine
                    w = (qc_lo - 2 * qp) * 512 + off
                    nc.vector.tensor_mul(expt[:, w:w + 128],
                                         expt[:, w:w + 128], mask01[:])
                return expt

            def pv_step(h, qp, t, expt):
                qcs = (2 * qp, 2 * qp + 1)
                qc_lo = t // 4
                off = 128 * (t % 4)
                for half, qc in enumerate(qcs):
                    if qc < qc_lo:
                        continue
                    cs = off if qc == qc_lo else 0
                    nc.tensor.matmul(
                        pouts[(h, qc)][:, cs:512],
                        vp[:, h, t, :],
                        expt[:, half * 512 + cs:(half + 1) * 512],
                        start=(t == 0), stop=(t == 4 * qc + 3),
                    )

            def divide(h, qc):
                # evict the finished chain at once so its PSUM bank frees
                # immediately; the slow recip/divide runs off the copy
                pout = pouts.pop((h, qc))
                nout = sb_div.tile([DK, 512], F32, tag="nout",
                                   name=f"no{h}_{qc}")
                nc.vector.tensor_copy(nout[:], pout[0:DK, :])
                den = sb_div.tile([1, 512], F32, tag="den",
                                  name=f"de{h}_{qc}")
                nc.vector.tensor_copy(den[:], pout[DK:DK + 1, :])
                # broadcast the denominator over 64 partitions FIRST, then
                # take the reciprocal 64 lanes wide: plain reciprocal is
                # ~23ns/lane-element (iterative), and approx_fast (~51 ULP,
                # plenty for a softmax denominator >= 1) is one DVE op
                dbc = sb_div.tile([DK, 512], F32, tag="dbc",
                                  name=f"db{h}_{qc}")
                nc.gpsimd.partition_broadcast(dbc[:], den[:])
                prb = sb_div.tile([DK, 512], F32, tag="prb",
                                  name=f"prb{h}_{qc}")
                nc.vector.reciprocal_approx_fast(prb[:], dbc[:])
                if h == 1:
                    # h1 lands at partitions 64-127: shift via SBUF DMA
                    tmp = sb_div.tile([DK, 512], FP16, tag="tmp",
                                      name=f"tmp{h}_{qc}")
                    nc.vector.tensor_mul(tmp[:], nout[0:DK, :], prb[:])
                    nc.sync.dma_start(
                        oct_[DK:128, 0, qc * 512:(qc + 1) * 512], tmp[:])
                else:
                    nc.vector.tensor_mul(
                        oct_[0:DK, h // 2, qc * 512:(qc + 1) * 512],
                        nout[0:DK, :], prb[:],
                    )

            def d_proj(qt):
                # output projection for one 128-row q-tile, split into two
                # single-bank psum tiles so it can borrow the filler pool
                pa = ps_f.tile([128, 512], F32, tag="fil", name=f"pa{qt}")
                pb = ps_f.tile([128, 256], F32, tag="fil", name=f"pb{qt}")
                for c, kk in ((0, 128), (1, DK)):
                    nc.tensor.matmul(pa[:], oct_[0:kk, c, qt * 128:(qt + 1) * 128],
                                     wot[0:kk, c, 0:512],
                                     start=(c == 0), stop=(c == 1))
                for c, kk in ((0, 128), (1, DK)):
                    nc.tensor.matmul(pb[:], oct_[0:kk, c, qt * 128:(qt + 1) * 128],
                                     wot[0:kk, c, 512:D],
                                     start=(c == 0), stop=(c == 1))
                ot = sb_exp.tile([128, D], FP16, tag="exp", name=f"ot{qt}")
                nc.vector.tensor_copy(ot[:, 0:512], pa[:])
                nc.vector.tensor_copy(ot[:, 512:D], pb[:])
                nc.sync.dma_start(out_d[qt * 128:(qt + 1) * 128, :], ot[:])

            # PE filler work, scheduled at fixed pipeline positions k:
            #   region 1 (k 0..23, attention q-half 0): QKV chains + V'
            #   transposes of s-chunks 2,3 (sc3's Q chunks m0/m2 must land
            #   before k=24 when q-half-1 scores start).
            #   region 2: projection of q-chunks 0,1 once all heads' divides
            #   for them completed (k>=26 / k>=40); q-chunk 2 near the end.
            fills = {}
            for n, m in enumerate(range(NM)):
                fills.setdefault(1 + 2 * n, []).append(
                    lambda m=m: qkv_chain(ps_f, "fil", 2, m))
            for h in range(HPC):
                fills.setdefault(11 + h, []).append(
                    lambda h=h: v_transposes(ps_f, "fil", 2, h))
            # sc3: Q chunks (m0/m2) must land before k=24; V-carrying chunks
            # (m2/m3/m4) before their transposes; only m1 (pure K, unread
            # until k~36) shifts into region 2 where the PE has slack
            for n, m in enumerate((0, 2, 4, 3)):
                fills.setdefault(14 + 2 * n, []).append(
                    lambda m=m: qkv_chain(ps_f, "fil", 3, m))
            for h in range(HPC):
                fills.setdefault(21 + h, []).append(
                    lambda h=h: v_transposes(ps_f, "fil", 3, h))
            for n, m in enumerate((1,)):
                fills.setdefault(26 + 2 * n, []).append(
                    lambda m=m: qkv_chain(ps_f, "fil", 3, m))
            for n, qt in enumerate(range(0, 4)):        # q-chunk 0
                fills.setdefault(30 + 3 * n, []).append(
                    lambda qt=qt: d_proj(qt))
            for n, qt in enumerate(range(4, 8)):        # q-chunk 1
                fills.setdefault(42 + 3 * n, []).append(
                    lambda qt=qt: d_proj(qt))
            for n, qt in enumerate(range(8, 12)):       # q-chunk 2
                fills.setdefault(70 + n, []).append(
                    lambda qt=qt: d_proj(qt))

            # Flat pipelined stream over all (head, q-half, k-tile) steps,
            # q-half-major so region 1 only needs s-chunks 0,1. Scores run
            # R=2 ahead of PV (matching the 2-slot score pool) so the PE
            # always has queued matmuls while ACT computes the exps.
            sc_list = [(h, 0, t) for h in range(HPC) for t in range(8)] + \
                      [(h, 1, t) for h in range(HPC) for t in range(16)]
            R = 2
            expts = {}
            for k in range(len(sc_list) + R):
                if k < len(sc_list):
                    expts[sc_list[k]] = score_step(*sc_list[k])
                j = k - R
                if j >= 0:
                    h, qp, t = sc_list[j]
                    if t == 0:
                        for qc in (2 * qp, 2 * qp + 1):
                            pouts[(h, qc)] = ps_o.tile([DK + 1, 512], F32,
                                                       tag="pout",
                                                       name=f"po{h}_{qc}")
                    pv_step(h, qp, t, expts.pop(sc_list[j]))
                    if t == 4 * (2 * qp) + 3:
                        divide(h, 2 * qp)
                    if t == 4 * (2 * qp + 1) + 3:
                        divide(h, 2 * qp + 1)
                for fn in fills.pop(k, ()):
                    fn()

            for fns in sorted(fills):
                for fn in fills[fns]:
                    fn()
            for qt in range(12, 16):                    # q-chunk 3 trails
                d_proj(qt)

    nc.compile()
    _NC_CACHE[key] = nc
    return nc


def make_in_maps(X, Wq, Wk, Wv, Wo):
    X = np.ascontiguousarray(np.asarray(X, dtype=np.float32))
    Wq = np.asarray(Wq, dtype=np.float32)
    Wk = np.asarray(Wk, dtype=np.float32)
    Wv = np.asarray(Wv, dtype=np.float32)
    Wo = np.asarray(Wo, dtype=np.float32)

    # causal 0/1 window mask: keep q >= k; rows=k (p), cols=q (f)
    p = np.arange(128)[:, None]
    f = np.arange(128)[None, :]
    mask = (f >= p).astype(np.float16)
    ident = np.eye(128, dtype=np.float32)

    in_maps = []
    for c in range(NCORES):
        b, hg = c // 4, c % 4
        gh = [hg * HPC + l for l in range(HPC)]
        q = [Wq[g * DK:(g + 1) * DK, :] / 8.0 for g in gh]
        k = [Wk[g * DK:(g + 1) * DK, :] for g in gh]
        v = [Wv[g * DK:(g + 1) * DK, :] for g in gh]
        wcat_rows = np.vstack([
            q[0], q[1], k[0], k[1], q[2], v[0], k[2], v[1], v[2],
            np.zeros((DK, D), dtype=np.float32),
        ])                                            # (640, 768)
        wcat = np.ascontiguousarray(wcat_rows.T.reshape(NI, 128, NM * 128))
        w0, w1, w2 = (Wo[:, g * DK:(g + 1) * DK].T for g in gh)
        wot = np.ascontiguousarray(np.stack([
            np.vstack([w0, w1]),
            np.vstack([w2, np.zeros((DK, D), dtype=np.float32)]),
        ]).astype(np.float16))                                     # (2, 128, 768)
        xt = np.ascontiguousarray(X[b].T.reshape(NI, 128, S))
        in_maps.append({
            "xt": xt, "wcat": wcat, "wot": wot,
            "mask": mask, "ident": ident,
        })
    return in_maps


def _run(in_maps, trace=False, trace_cores=None):
    nc = build_nc()
    return bass_utils.run_bass_kernel_spmd(
        nc, in_maps, core_ids=list(range(NCORES)),
        trace=trace, trace_cores=trace_cores,
    )


def kernel(X, Wq, Wk, Wv, Wo):
    in_maps = make_in_maps(X, Wq, Wk, Wv, Wo)
    res = _run(in_maps, trace=False)
    out = np.zeros((B, S, D), dtype=np.float32)
    for c in range(NCORES):
        out[c // 4] += res.results[c]["out"]
    return out


# revision 20
# speedup vs baseline: 1.2320x; 1.1045x over previous
"""Causal multi-head self-attention (B=2, S=2048, D=768, H=12) on 8 TRN2 NeuronCores.

Sharding: core c = (batch b=c//4, head-group hg=c%4 of 3 heads).
Each core computes Q/K/V for its 3 heads, causal attention, and the partial
output projection sum_h out_h @ Wo[:, h]^T -> (S, D) in fp16. Host sums the
4 head-group partials per batch (the unshard step).

On-core dataflow (transposed (feature, seq) layout, f32r matmuls), arranged
as one globally-woven instruction stream that keeps TensorE dense (the HAM
clock-gate re-throttles the PE to 1.2 GHz after any ~3.4us window with idle
gaps, which doubles every matmul):

  region 0: QKV^T chains for s-chunks 0,1 (psum[m,s] += WcatT[i,m].T @ XT[i,s]
     per 512-col chunk, 8-bank rotation) + V' transposes. DMA descriptors for
     X/weights are generated on sync+scalar+gpsimd in parallel, first chunk
     prioritized.
  region 1: attention q-half 0 for all 3 heads (24 pipelined steps), woven
     with the QKV chains + V' transposes of s-chunks 2,3 (the PE filler that
     keeps it busy while ACT exps).
  region 2: attention q-half 1 (48 steps), woven with output projection
     blocks for q-chunks 0,1 as their divides complete; projection of
     q-chunks 2,3 trails at the end.

Attention step pipeline (scores run R=2 steps ahead of PV):
  scoresT[k,q] = KT.T @ QT per 1024-wide q-half -> exp on ACT -> fp16 expt
  (NO pre-exp mask: the diagonal window is zeroed AFTER exp by a DVE
  multiply with a 0/1 fp16 mask, so ACT never waits on DVE) ->
  PV: pout[qc] += V'[t].T @ expT (65 rows: 64 data + denominator).
  Per-(h,qc) epilogue: recip(den) -> broadcast -> numerator * recip.

PSUM: scores (128,1024)=2 banks x2 + 2 pout banks + 2 filler banks = 8.
"""

import numpy as np
from contextlib import ExitStack

import concourse.bass as bass
import concourse.tile as tile
from concourse import bacc, mybir
from concourse import bass_utils

F32 = mybir.dt.float32
F32R = mybir.dt.float32r
BF16 = mybir.dt.bfloat16
FP16 = mybir.dt.float16
AF = mybir.ActivationFunctionType

B, S, D, H = 2, 2048, 768, 12
DK = 64
HPC = 3            # heads per core
NCORES = 8
NI = D // 128      # 6 input-feature chunks
NM = 5             # output m-chunks of 128 (640 rows incl. 64 pad)
NT = S // 128      # 16 k-tiles
NQC = S // 512     # 4 q-chunks

# per-local-head (base_partition, m_chunk) in the QKVT buffer
QPOS = [(0, 0), (64, 0), (0, 2)]
KPOS = [(0, 1), (64, 1), (0, 3)]
VPOS = [(64, 2), (64, 3), (0, 4)]

_NC_CACHE = {}


def build_nc(dbg=False):
    key = ("nc", dbg)
    if key in _NC_CACHE:
        return _NC_CACHE[key]
    nc = bacc.Bacc("TRN2", target_bir_lowering=False, debug=False,
                   num_devices=NCORES)

    xt_d = nc.dram_tensor("xt", [NI, 128, S], F32R, kind="ExternalInput").ap()
    wcat_d = nc.dram_tensor("wcat", [NI, 128, NM * 128], F32R, kind="ExternalInput").ap()
    wot_d = nc.dram_tensor("wot", [2, 128, D], FP16, kind="ExternalInput").ap()
    mask_d = nc.dram_tensor("mask", [128, 128], FP16, kind="ExternalInput").ap()
    id_d = nc.dram_tensor("ident", [128, 128], F32R, kind="ExternalInput").ap()
    out_d = nc.dram_tensor("out", [S, D], FP16, kind="ExternalOutput").ap()

    with tile.TileContext(nc) as tc, ExitStack() as ctx:
        const = ctx.enter_context(tc.tile_pool(name="const", bufs=1))

        # persistent SBUF buffers
        xt = const.tile([128, NI, S], F32R)             # X^T
        wcat = const.tile([128, NI, NM * 128], F32R)    # W^T (QKV packed)
        wot = const.tile([128, 2, D], FP16)             # Wo^T [h0;h1],[h2;pad]
        mask01 = const.tile([128, 128], FP16)           # 0/1 causal window mask
        ident = const.tile([128, 128], F32R)
        qkvt = const.tile([128, NM, S], F32R)           # Q^T/K^T/V^T packed
        vp = const.tile([128, HPC, NT, DK + 1], FP16)   # V' = [V | ones]
        oct_ = const.tile([128, 2, S], FP16)            # packed out^T [h0;h1],[h2]
        qk16 = const.tile([128, 4, S], FP16)            # fp16 Q/K for attention
        junk = const.tile([128, 512], FP16)             # PE warmup fodder

        # DMA issue order = arrival priority. Spread descriptor generation
        # over three otherwise-idle engine queues; region 0's critical path
        # (wcat m-chunk-0 columns + the first s-chunk of X) goes first.
        ENGS = (nc.sync, nc.scalar, nc.gpsimd)
        for i in range(NI):
            ENGS[i % 3].dma_start(wcat[:, i, 0:128], wcat_d[i][:, 0:128])
        for i in range(NI):
            ENGS[i % 3].dma_start(xt[:, i, 0:512], xt_d[i][:, 0:512])
        for i in range(NI):
            ENGS[i % 3].dma_start(wcat[:, i, 128:NM * 128],
                                  wcat_d[i][:, 128:NM * 128])
        nc.sync.dma_start(ident[:], id_d)
        nc.vector.memset(vp[:, :, :, DK:DK + 1], 1.0)   # denominator ones col
        nc.vector.memset(junk[:], 1.0)
        for i in range(NI):
            ENGS[i % 3].dma_start(xt[:, i, 512:1024], xt_d[i][:, 512:1024])
        nc.gpsimd.dma_start(mask01[:], mask_d)
        for sc in range(2, NQC):
            for i in range(NI):
                ENGS[(sc * NI + i) % 3].dma_start(
                    xt[:, i, sc * 512:(sc + 1) * 512],
                    xt_d[i][:, sc * 512:(sc + 1) * 512])
        nc.sync.dma_start(wot[:], wot_d.rearrange("c p f -> p c f"))

        # ---- QKV^T projection chain + V' transpose emitters (shared by
        # region 0 and the weave)
        def qkv_chain(pool, tag, sc, m):
            s0 = sc * 512
            pq = pool.tile([128, 512], F32, tag=tag, name=f"pq{sc}_{m}")
            for i in range(NI):
                nc.tensor.matmul(
                    pq[:],
                    wcat[:, i, m * 128:(m + 1) * 128],
                    xt[:, i, s0:s0 + 512],
                    start=(i == 0), stop=(i == NI - 1),
                )
            nc.vector.tensor_copy(qkvt[:, m, s0:s0 + 512], pq[:])
            # fp16 shadow of Q/K rows for the attention core
            if m <= 1:
                nc.vector.tensor_copy(qk16[:, m, s0:s0 + 512], pq[:])
            elif m <= 3:
                nc.vector.tensor_copy(qk16[0:DK, m, s0:s0 + 512], pq[0:DK, :])

        def v_transposes(pool, tag, sc, h):
            vb, vchunk = VPOS[h]
            for t in range(4 * sc, 4 * sc + 4):
                ptr = pool.tile([128, DK], F32R, tag=tag, name=f"tr{h}_{t}")
                nc.tensor.transpose(
                    ptr[:],
                    qkvt[vb:vb + DK, vchunk, t * 128:(t + 1) * 128],
                    ident[vb:vb + DK, vb:vb + DK],
                )
                nc.vector.tensor_copy(vp[:, h, t, 0:DK], ptr[:])

        # ---- region 0: the minimal chain prefix attention needs (all of
        # s-chunk 0 plus sc1's Q/K/V0 chunks m0-m2; sc1's m3/m4 weave into
        # region 1), preceded by a warmup burst of junk matmuls during the
        # DMA lead-in so the HAM clock-gate reaches 2.4 GHz before the
        # first real chain issues
        with tc.tile_pool(name="ps_ab", bufs=7, space="PSUM") as ps_ab:
            wtile = ps_ab.tile([128, 512], F32, tag="warm", bufs=1,
                               name="warm")
            for w in range(30):
                nc.tensor.matmul(wtile[:], junk[:, 0:128], junk[:],
                                 start=True, stop=True)
            for m in range(NM):
                qkv_chain(ps_ab, "ab", 0, m)
            for m in (0, 1, 2):
                qkv_chain(ps_ab, "ab", 1, m)

        # ---- regions 1+2: woven attention + trailing QKV + projection
        with tc.tile_pool(name="ps_s", bufs=2, space="PSUM") as ps_s, \
             tc.tile_pool(name="ps_o", bufs=2, space="PSUM") as ps_o, \
             tc.tile_pool(name="ps_f", bufs=2, space="PSUM") as ps_f, \
             tc.tile_pool(name="sb_exp", bufs=6) as sb_exp, \
             tc.tile_pool(name="sb_div", bufs=3) as sb_div:

            pouts = {}

            def score_step(h, qp, t):
                qb, qchunk = QPOS[h]
                kb, kchunk = KPOS[h]
                qcs = (2 * qp, 2 * qp + 1)
                qc_lo = t // 4
                off = 128 * (t % 4)   # diag col offset inside qc_lo's half
                pscr = ps_s.tile([128, 1024], F32, tag="scr",
                                 name=f"sc{h}_{qp}_{t}")
                for half, qc in enumerate(qcs):
                    if qc < qc_lo:
                        continue
                    cs = off if qc == qc_lo else 0  # skip fully-masked cols
                    nc.tensor.matmul(
                        pscr[:, half * 512 + cs:(half + 1) * 512],
                        qk16[kb:kb + DK, kchunk, t * 128:(t + 1) * 128],
                        qk16[qb:qb + DK, qchunk,
                             qc * 512 + cs:(qc + 1) * 512],
                        start=True, stop=True,
                    )
                lo = (512 if qc_lo == qcs[1] else 0) + \
                     (off if qc_lo in qcs else 0)
                expt = sb_exp.tile([128, 1024], FP16, tag="exp",
                                   name=f"ex{h}_{qp}_{t}")
                nc.scalar.activation(expt[:, lo:1024], pscr[:, lo:1024],
                                     AF.Exp)
                if qc_lo in qcs:
                    # zero the above-diagonal weights AFTER exp (0/1 fp16
                    # mask on DVE) so ACT never waits on another engine
                    w = (qc_lo - 2 * qp) * 512 + off
                    nc.vector.tensor_mul(expt[:, w:w + 128],
                                         expt[:, w:w + 128], mask01[:])
                return expt

            def pv_step(h, qp, t, expt):
                qcs = (2 * qp, 2 * qp + 1)
                qc_lo = t // 4
                off = 128 * (t % 4)
                for half, qc in enumerate(qcs):
                    if qc < qc_lo:
                        continue
                    cs = off if qc == qc_lo else 0
                    nc.tensor.matmul(
                        pouts[(h, qc)][:, cs:512],
                        vp[:, h, t, :],
                        expt[:, half * 512 + cs:(half + 1) * 512],
                        start=(t == 0), stop=(t == 4 * qc + 3),
                    )

            def divide(h, qc):
                # evict the finished chain at once so its PSUM bank frees
                # immediately; the slow recip/divide runs off the copy
                pout = pouts.pop((h, qc))
                nout = sb_div.tile([DK + 1, 512], F32, tag="nout",
                                   name=f"no{h}_{qc}")
                nc.vector.tensor_copy(nout[:], pout[:])
                # spread the 512-wide den row over 64 partitions so the
                # expensive (~23ns/lane-elem) reciprocal runs 8-deep per
                # lane, not 512
                rsp = sb_div.tile([DK, 8], F32, tag="rsp",
                                  name=f"rsp{h}_{qc}")
                nc.sync.dma_start(rsp[:], nout[DK:DK + 1, :])
                rcs = sb_div.tile([DK, 8], F32, tag="rcs",
                                  name=f"rcs{h}_{qc}")
                nc.vector.reciprocal(rcs[:], rsp[:])
                rc0 = sb_div.tile([1, 512], F32, tag="rc0",
                                  name=f"rc0{h}_{qc}")
                nc.sync.dma_start(rc0[:], rcs[:])
                prb = sb_div.tile([DK, 512], F32, tag="prb",
                                  name=f"prb{h}_{qc}")
                nc.gpsimd.partition_broadcast(prb[:], rc0[:])
                if h == 1:
                    # h1 lands at partitions 64-127: shift via SBUF DMA
                    tmp = sb_div.tile([DK, 512], FP16, tag="tmp",
                                      name=f"tmp{h}_{qc}")
                    nc.vector.tensor_mul(tmp[:], nout[0:DK, :], prb[:])
                    nc.sync.dma_start(
                        oct_[DK:128, 0, qc * 512:(qc + 1) * 512], tmp[:])
                else:
                    nc.vector.tensor_mul(
                        oct_[0:DK, h // 2, qc * 512:(qc + 1) * 512],
                        nout[0:DK, :], prb[:],
                    )

            def d_proj(qt):
                # output projection for one 128-row q-tile, split into two
                # single-bank psum tiles so it can borrow the filler pool
                pa = ps_f.tile([128, 512], F32, tag="fil", name=f"pa{qt}")
                pb = ps_f.tile([128, 256], F32, tag="fil", name=f"pb{qt}")
                for c, kk in ((0, 128), (1, DK)):
                    nc.tensor.matmul(pa[:], oct_[0:kk, c, qt * 128:(qt + 1) * 128],
                                     wot[0:kk, c, 0:512],
                                     start=(c == 0), stop=(c == 1))
                for c, kk in ((0, 128), (1, DK)):
                    nc.tensor.matmul(pb[:], oct_[0:kk, c, qt * 128:(qt + 1) * 128],
                                     wot[0:kk, c, 512:D],
                                     start=(c == 0), stop=(c == 1))
                ot = sb_exp.tile([128, D], FP16, tag="exp", name=f"ot{qt}")
                nc.vector.tensor_copy(ot[:, 0:512], pa[:])
                nc.vector.tensor_copy(ot[:, 512:D], pb[:])
                nc.sync.dma_start(out_d[qt * 128:(qt + 1) * 128, :], ot[:])

            # PE filler work, scheduled at fixed pipeline positions k:
            #   region 1 (k 0..23, attention q-half 0): QKV chains + V'
            #   transposes of s-chunks 2,3 (sc3's Q chunks m0/m2 must land
            #   before k=24 when q-half-1 scores start).
            #   region 2: projection of q-chunks 0,1 once all heads' divides
            #   for them completed (k>=26 / k>=40); q-chunk 2 near the end.
            # Fill schedule invariants: a filler must be EMITTED before its
            # first reader's pipeline position AND after its own inputs'
            # positions (Tile only tracks read-after-prior-write; emitting a
            # reader before its writer reads stale SBUF).
            #   pv(h,qp,t) sits at k = R + head-offset + 16*qp + t
            #   V' transposes for (h, sc) need the V-carrying chain of sc
            #   (m2->h0, m3->h1, m4->h2) and feed pv tiles t=4sc..4sc+3.
            fills = {}
            fills[0] = [lambda: v_transposes(ps_f, "fil", 0, 0)]
            fills[1] = [lambda: v_transposes(ps_f, "fil", 1, 0)]
            fills[2] = [lambda: qkv_chain(ps_f, "fil", 1, 3)]
            fills[3] = [lambda: qkv_chain(ps_f, "fil", 1, 4)]
            fills[4] = [lambda: v_transposes(ps_f, "fil", 0, 1)]
            fills[5] = [lambda: v_transposes(ps_f, "fil", 1, 1)]
            fills[6] = [lambda: v_transposes(ps_f, "fil", 0, 2)]
            fills[7] = [lambda: v_transposes(ps_f, "fil", 1, 2)]
            for n, m in enumerate(range(NM)):
                fills.setdefault(8 + 2 * n, []).append(
                    lambda m=m: qkv_chain(ps_f, "fil", 2, m))
            fills[17] = [lambda: qkv_chain(ps_f, "fil", 3, 0)]
            fills[18] = [lambda: qkv_chain(ps_f, "fil", 3, 2)]
            fills[19] = [lambda: v_transposes(ps_f, "fil", 2, 0)]
            fills[20] = [lambda: qkv_chain(ps_f, "fil", 3, 4)]
            fills[21] = [lambda: v_transposes(ps_f, "fil", 2, 1)]
            fills[22] = [lambda: qkv_chain(ps_f, "fil", 3, 3)]
            fills[23] = [lambda: v_transposes(ps_f, "fil", 2, 2)]
            fills[24] = [lambda: v_transposes(ps_f, "fil", 3, 0)]
            fills[25] = [lambda: v_transposes(ps_f, "fil", 3, 1)]
            fills[26] = [lambda: v_transposes(ps_f, "fil", 3, 2)]
            fills[28] = [lambda: qkv_chain(ps_f, "fil", 3, 1)]
            for n, qt in enumerate(range(0, 4)):        # q-chunk 0
                fills.setdefault(30 + 3 * n, []).append(
                    lambda qt=qt: d_proj(qt))
            for n, qt in enumerate(range(4, 8)):        # q-chunk 1
                fills.setdefault(42 + 3 * n, []).append(
                    lambda qt=qt: d_proj(qt))
            for n, qt in enumerate(range(8, 12)):       # q-chunk 2
                fills.setdefault(70 + n, []).append(
                    lambda qt=qt: d_proj(qt))

            # Flat pipelined stream over all (head, q-half, k-tile) steps,
            # q-half-major so region 1 only needs s-chunks 0,1. Scores run
            # R=2 ahead of PV (matching the 2-slot score pool) so the PE
            # always has queued matmuls while ACT computes the exps.
            sc_list = [(h, 0, t) for h in range(HPC) for t in range(8)] + \
                      [(h, 1, t) for h in range(HPC) for t in range(16)]
            R = 2
            expts = {}
            for k in range(len(sc_list) + R):
                if k < len(sc_list):
                    expts[sc_list[k]] = score_step(*sc_list[k])
                j = k - R
                if j >= 0:
                    h, qp, t = sc_list[j]
                    if t == 0:
                        for qc in (2 * qp, 2 * qp + 1):
                            pouts[(h, qc)] = ps_o.tile([DK + 1, 512], F32,
                                                       tag="pout",
                                                       name=f"po{h}_{qc}")
                    pv_step(h, qp, t, expts.pop(sc_list[j]))
                    if t == 4 * (2 * qp) + 3:
                        divide(h, 2 * qp)
                    if t == 4 * (2 * qp + 1) + 3:
                        divide(h, 2 * qp + 1)
                for fn in fills.pop(k, ()):
                    fn()

            for fns in sorted(fills):
                for fn in fills[fns]:
                    fn()
            for qt in range(12, 16):                    # q-chunk 3 trails
                d_proj(qt)

    nc.compile()
    _NC_CACHE[key] = nc
    return nc


def make_in_maps(X, Wq, Wk, Wv, Wo):
    X = np.ascontiguousarray(np.asarray(X, dtype=np.float32))
    Wq = np.asarray(Wq, dtype=np.float32)
    Wk = np.asarray(Wk, dtype=np.float32)
    Wv = np.asarray(Wv, dtype=np.float32)
    Wo = np.asarray(Wo, dtype=np.float32)

    # causal 0/1 window mask: keep q >= k; rows=k (p), cols=q (f)
    p = np.arange(128)[:, None]
    f = np.arange(128)[None, :]
    mask = (f >= p).astype(np.float16)
    ident = np.eye(128, dtype=np.float32)

    in_maps = []
    for c in range(NCORES):
        b, hg = c // 4, c % 4
        gh = [hg * HPC + l for l in range(HPC)]
        q = [Wq[g * DK:(g + 1) * DK, :] / 8.0 for g in gh]
        k = [Wk[g * DK:(g + 1) * DK, :] for g in gh]
        v = [Wv[g * DK:(g + 1) * DK, :] for g in gh]
        wcat_rows = np.vstack([
            q[0], q[1], k[0], k[1], q[2], v[0], k[2], v[1], v[2],
            np.zeros((DK, D), dtype=np.float32),
        ])                                            # (640, 768)
        wcat = np.ascontiguousarray(wcat_rows.T.reshape(NI, 128, NM * 128))
        w0, w1, w2 = (Wo[:, g * DK:(g + 1) * DK].T for g in gh)
        wot = np.ascontiguousarray(np.stack([
            np.vstack([w0, w1]),
            np.vstack([w2, np.zeros((DK, D), dtype=np.float32)]),
        ]).astype(np.float16))                                     # (2, 128, 768)
        xt = np.ascontiguousarray(X[b].T.reshape(NI, 128, S))
        in_maps.append({
            "xt": xt, "wcat": wcat, "wot": wot,
            "mask": mask, "ident": ident,
        })
    return in_maps


def _run(in_maps, trace=False, trace_cores=None):
    nc = build_nc()
    return bass_utils.run_bass_kernel_spmd(
        nc, in_maps, core_ids=list(range(NCORES)),
        trace=trace, trace_cores=trace_cores,
    )


def kernel(X, Wq, Wk, Wv, Wo):
    in_maps = make_in_maps(X, Wq, Wk, Wv, Wo)
    res = _run(in_maps, trace=False)
    out = np.zeros((B, S, D), dtype=np.float32)
    for c in range(NCORES):
        out[c // 4] += res.results[c]["out"]
    return out


# revision 25
# speedup vs baseline: 1.3262x; 1.0765x over previous
"""Causal multi-head self-attention (B=2, S=2048, D=768, H=12) on 8 TRN2 NeuronCores.

Sharding: core c = (batch b=c//4, head-group hg=c%4 of 3 heads).
Each core computes Q/K/V for its 3 heads, causal attention, and the partial
output projection sum_h out_h @ Wo[:, h]^T -> (S, D) in fp16. Host sums the
4 head-group partials per batch (the unshard step).

On-core dataflow (transposed (feature, seq) layout, f32r matmuls), arranged
as one globally-woven instruction stream that keeps TensorE dense (the HAM
clock-gate re-throttles the PE to 1.2 GHz after any ~3.4us window with idle
gaps, which doubles every matmul):

  region 0: QKV^T chains for s-chunks 0,1 (psum[m,s] += WcatT[i,m].T @ XT[i,s]
     per 512-col chunk, 8-bank rotation) + V' transposes. DMA descriptors for
     X/weights are generated on sync+scalar+gpsimd in parallel, first chunk
     prioritized.
  region 1: attention q-half 0 for all 3 heads (24 pipelined steps), woven
     with the QKV chains + V' transposes of s-chunks 2,3 (the PE filler that
     keeps it busy while ACT exps).
  region 2: attention q-half 1 (48 steps), woven with output projection
     blocks for q-chunks 0,1 as their divides complete; projection of
     q-chunks 2,3 trails at the end.

Attention step pipeline (scores run R=2 steps ahead of PV):
  scoresT[k,q] = KT.T @ QT per 1024-wide q-half -> exp on ACT -> fp16 expt
  (NO pre-exp mask: the diagonal window is zeroed AFTER exp by a DVE
  multiply with a 0/1 fp16 mask, so ACT never waits on DVE) ->
  PV: pout[qc] += V'[t].T @ expT (65 rows: 64 data + denominator).
  Per-(h,qc) epilogue: recip(den) -> broadcast -> numerator * recip.

PSUM: scores (128,1024)=2 banks x2 + 2 pout banks + 2 filler banks = 8.
"""

import ml_dtypes
import numpy as np
from contextlib import ExitStack

import concourse.bass as bass
import concourse.tile as tile
from concourse import bacc, mybir
from concourse import bass_utils

F32 = mybir.dt.float32
F32R = mybir.dt.float32r
BF16 = mybir.dt.bfloat16
FP16 = mybir.dt.float16
AF = mybir.ActivationFunctionType

B, S, D, H = 2, 2048, 768, 12
DK = 64
HPC = 3            # heads per core
NCORES = 8
NI = D // 128      # 6 input-feature chunks
NM = 5             # output m-chunks of 128 (640 rows incl. 64 pad)
NT = S // 128      # 16 k-tiles
NQC = S // 512     # 4 q-chunks

# per-local-head (base_partition, m_chunk) in the QKVT buffer
QPOS = [(0, 0), (64, 0), (0, 2)]
KPOS = [(0, 1), (64, 1), (0, 3)]
VPOS = [(64, 2), (64, 3), (0, 4)]

_NC_CACHE = {}


def build_nc(dbg=False):
    key = ("nc", dbg)
    if key in _NC_CACHE:
        return _NC_CACHE[key]
    nc = bacc.Bacc("TRN2", target_bir_lowering=False, debug=False,
                   num_devices=NCORES)

    xt_d = nc.dram_tensor("xt", [NI, 128, S], BF16, kind="ExternalInput").ap()
    wcat_d = nc.dram_tensor("wcat", [NI, 128, NM * 128], BF16, kind="ExternalInput").ap()
    wot_d = nc.dram_tensor("wot", [2, 128, D], FP16, kind="ExternalInput").ap()
    mask_d = nc.dram_tensor("mask", [128, 128], FP16, kind="ExternalInput").ap()
    id_d = nc.dram_tensor("ident", [128, 128], F32R, kind="ExternalInput").ap()
    out_d = nc.dram_tensor("out", [S, D], FP16, kind="ExternalOutput").ap()

    with tile.TileContext(nc) as tc, ExitStack() as ctx:
        const = ctx.enter_context(tc.tile_pool(name="const", bufs=1))

        # persistent SBUF buffers
        xt = const.tile([128, NI, S], BF16)             # X^T (bf16: halves DMA)
        wcat = const.tile([128, NI, NM * 128], BF16)    # W^T (QKV packed)
        wot = const.tile([128, 2, D], FP16)             # Wo^T [h0;h1],[h2;pad]
        mask01 = const.tile([128, 128], FP16)           # 0/1 causal window mask
        ident = const.tile([128, 128], F32R)
        qkvt = const.tile([128, NM, S], F32R)           # Q^T/K^T/V^T packed
        vp = const.tile([128, HPC, NT, DK + 1], FP16)   # V' = [V | ones]
        oct_ = const.tile([128, 2, S], FP16)            # packed out^T [h0;h1],[h2]
        qk16 = const.tile([128, 4, S], FP16)            # fp16 Q/K for attention
        junk = const.tile([128, 512], FP16)             # PE warmup fodder

        # DMA issue order = arrival priority. Spread descriptor generation
        # over three otherwise-idle engine queues; region 0's critical path
        # (wcat m-chunk-0 columns + the first s-chunk of X) goes first.
        ENGS = (nc.sync, nc.scalar, nc.gpsimd)
        for i in range(NI):
            ENGS[i % 3].dma_start(wcat[:, i, 0:128], wcat_d[i][:, 0:128])
        for i in range(NI):
            ENGS[i % 3].dma_start(xt[:, i, 0:512], xt_d[i][:, 0:512])
        for i in range(NI):
            ENGS[i % 3].dma_start(wcat[:, i, 128:NM * 128],
                                  wcat_d[i][:, 128:NM * 128])
        nc.sync.dma_start(ident[:], id_d)
        nc.vector.memset(vp[:, :, :, DK:DK + 1], 1.0)   # denominator ones col
        nc.vector.memset(junk[:], 1.0)
        for i in range(NI):
            ENGS[i % 3].dma_start(xt[:, i, 512:1024], xt_d[i][:, 512:1024])
        nc.gpsimd.dma_start(mask01[:], mask_d)
        for sc in range(2, NQC):
            for i in range(NI):
                ENGS[(sc * NI + i) % 3].dma_start(
                    xt[:, i, sc * 512:(sc + 1) * 512],
                    xt_d[i][:, sc * 512:(sc + 1) * 512])
        nc.sync.dma_start(wot[:], wot_d.rearrange("c p f -> p c f"))

        # ---- QKV^T projection chain + V' transpose emitters (shared by
        # region 0 and the weave)
        def qkv_chain(pool, tag, sc, m):
            s0 = sc * 512
            pq = pool.tile([128, 512], F32, tag=tag, name=f"pq{sc}_{m}")
            for i in range(NI):
                nc.tensor.matmul(
                    pq[:],
                    wcat[:, i, m * 128:(m + 1) * 128],
                    xt[:, i, s0:s0 + 512],
                    start=(i == 0), stop=(i == NI - 1),
                )
            nc.vector.tensor_copy(qkvt[:, m, s0:s0 + 512], pq[:])
            # fp16 shadow of Q/K rows for the attention core
            if m <= 1:
                nc.vector.tensor_copy(qk16[:, m, s0:s0 + 512], pq[:])
            elif m <= 3:
                nc.vector.tensor_copy(qk16[0:DK, m, s0:s0 + 512], pq[0:DK, :])

        def v_transposes(pool, tag, sc, h):
            vb, vchunk = VPOS[h]
            for t in range(4 * sc, 4 * sc + 4):
                ptr = pool.tile([128, DK], F32R, tag=tag, name=f"tr{h}_{t}")
                nc.tensor.transpose(
                    ptr[:],
                    qkvt[vb:vb + DK, vchunk, t * 128:(t + 1) * 128],
                    ident[vb:vb + DK, vb:vb + DK],
                )
                nc.vector.tensor_copy(vp[:, h, t, 0:DK], ptr[:])

        # ---- region 0: the minimal chain prefix attention needs (all of
        # s-chunk 0 plus sc1's Q/K/V0 chunks m0-m2; sc1's m3/m4 weave into
        # region 1), preceded by a warmup burst of junk matmuls during the
        # DMA lead-in so the HAM clock-gate reaches 2.4 GHz before the
        # first real chain issues
        with tc.tile_pool(name="ps_ab", bufs=7, space="PSUM") as ps_ab:
            wtile = ps_ab.tile([128, 512], F32, tag="warm", bufs=1,
                               name="warm")
            for w in range(30):
                nc.tensor.matmul(wtile[:], junk[:, 0:128], junk[:],
                                 start=True, stop=True)
            for m in range(NM):
                qkv_chain(ps_ab, "ab", 0, m)
            for m in (0, 1, 2):
                qkv_chain(ps_ab, "ab", 1, m)

        # ---- regions 1+2: woven attention + trailing QKV + projection
        with tc.tile_pool(name="ps_s", bufs=2, space="PSUM") as ps_s, \
             tc.tile_pool(name="ps_o", bufs=2, space="PSUM") as ps_o, \
             tc.tile_pool(name="ps_f", bufs=2, space="PSUM") as ps_f, \
             tc.tile_pool(name="sb_exp", bufs=6) as sb_exp, \
             tc.tile_pool(name="sb_div", bufs=3) as sb_div:

            pouts = {}

            def score_step(h, qp, t):
                qb, qchunk = QPOS[h]
                kb, kchunk = KPOS[h]
                qcs = (2 * qp, 2 * qp + 1)
                qc_lo = t // 4
                off = 128 * (t % 4)   # diag col offset inside qc_lo's half
                pscr = ps_s.tile([128, 1024], F32, tag="scr",
                                 name=f"sc{h}_{qp}_{t}")
                for half, qc in enumerate(qcs):
                    if qc < qc_lo:
                        continue
                    cs = off if qc == qc_lo else 0  # skip fully-masked cols
                    nc.tensor.matmul(
                        pscr[:, half * 512 + cs:(half + 1) * 512],
                        qk16[kb:kb + DK, kchunk, t * 128:(t + 1) * 128],
                        qk16[qb:qb + DK, qchunk,
                             qc * 512 + cs:(qc + 1) * 512],
                        start=True, stop=True,
                    )
                lo = (512 if qc_lo == qcs[1] else 0) + \
                     (off if qc_lo in qcs else 0)
                expt = sb_exp.tile([128, 1024], FP16, tag="exp",
                                   name=f"ex{h}_{qp}_{t}")
                nc.scalar.activation(expt[:, lo:1024], pscr[:, lo:1024],
                                     AF.Exp)
                if qc_lo in qcs:
                    # zero the above-diagonal weights AFTER exp (0/1 fp16
                    # mask on DVE) so ACT never waits on another engine
                    w = (qc_lo - 2 * qp) * 512 + off
                    nc.vector.tensor_mul(expt[:, w:w + 128],
                                         expt[:, w:w + 128], mask01[:])
                return expt

            def pv_step(h, qp, t, expt):
                qcs = (2 * qp, 2 * qp + 1)
                qc_lo = t // 4
                off = 128 * (t % 4)
                for half, qc in enumerate(qcs):
                    if qc < qc_lo:
                        continue
                    cs = off if qc == qc_lo else 0
                    nc.tensor.matmul(
                        pouts[(h, qc)][:, cs:512],
                        vp[:, h, t, :],
                        expt[:, half * 512 + cs:(half + 1) * 512],
                        start=(t == 0), stop=(t == 4 * qc + 3),
                    )

            def divide(h, qc):
                # evict the finished chain at once so its PSUM bank frees
                # immediately; the slow recip/divide runs off the copy
                pout = pouts.pop((h, qc))
                nout = sb_div.tile([DK + 1, 512], F32, tag="nout",
                                   name=f"no{h}_{qc}")
                nc.vector.tensor_copy(nout[:], pout[:])
                # spread the 512-wide den row over 64 partitions so the
                # expensive (~23ns/lane-elem) reciprocal runs 8-deep per
                # lane, not 512
                rsp = sb_div.tile([DK, 8], F32, tag="rsp",
                                  name=f"rsp{h}_{qc}")
                nc.sync.dma_start(rsp[:], nout[DK:DK + 1, :])
                rcs = sb_div.tile([DK, 8], F32, tag="rcs",
                                  name=f"rcs{h}_{qc}")
                nc.vector.reciprocal(rcs[:], rsp[:])
                rc0 = sb_div.tile([1, 512], F32, tag="rc0",
                                  name=f"rc0{h}_{qc}")
                nc.sync.dma_start(rc0[:], rcs[:])
                prb = sb_div.tile([DK, 512], F32, tag="prb",
                                  name=f"prb{h}_{qc}")
                nc.gpsimd.partition_broadcast(prb[:], rc0[:])
                if h == 1:
                    # h1 lands at partitions 64-127: shift via SBUF DMA
                    tmp = sb_div.tile([DK, 512], FP16, tag="tmp",
                                      name=f"tmp{h}_{qc}")
                    nc.vector.tensor_mul(tmp[:], nout[0:DK, :], prb[:])
                    nc.sync.dma_start(
                        oct_[DK:128, 0, qc * 512:(qc + 1) * 512], tmp[:])
                else:
                    nc.vector.tensor_mul(
                        oct_[0:DK, h // 2, qc * 512:(qc + 1) * 512],
                        nout[0:DK, :], prb[:],
                    )

            def d_proj(qt):
                # output projection for one 128-row q-tile, split into two
                # single-bank psum tiles so it can borrow the filler pool
                pa = ps_f.tile([128, 512], F32, tag="fil", name=f"pa{qt}")
                pb = ps_f.tile([128, 256], F32, tag="fil", name=f"pb{qt}")
                for c, kk in ((0, 128), (1, DK)):
                    nc.tensor.matmul(pa[:], oct_[0:kk, c, qt * 128:(qt + 1) * 128],
                                     wot[0:kk, c, 0:512],
                                     start=(c == 0), stop=(c == 1))
                for c, kk in ((0, 128), (1, DK)):
                    nc.tensor.matmul(pb[:], oct_[0:kk, c, qt * 128:(qt + 1) * 128],
                                     wot[0:kk, c, 512:D],
                                     start=(c == 0), stop=(c == 1))
                ot = sb_exp.tile([128, D], FP16, tag="exp", name=f"ot{qt}")
                nc.vector.tensor_copy(ot[:, 0:512], pa[:])
                nc.vector.tensor_copy(ot[:, 512:D], pb[:])
                nc.sync.dma_start(out_d[qt * 128:(qt + 1) * 128, :], ot[:])

            # PE filler work, scheduled at fixed pipeline positions k:
            #   region 1 (k 0..23, attention q-half 0): QKV chains + V'
            #   transposes of s-chunks 2,3 (sc3's Q chunks m0/m2 must land
            #   before k=24 when q-half-1 scores start).
            #   region 2: projection of q-chunks 0,1 once all heads' divides
            #   for them completed (k>=26 / k>=40); q-chunk 2 near the end.
            # Fill schedule invariants: a filler must be EMITTED before its
            # first reader's pipeline position AND after its own inputs'
            # positions (Tile only tracks read-after-prior-write; emitting a
            # reader before its writer reads stale SBUF).
            #   pv(h,qp,t) sits at k = R + head-offset + 16*qp + t
            #   V' transposes for (h, sc) need the V-carrying chain of sc
            #   (m2->h0, m3->h1, m4->h2) and feed pv tiles t=4sc..4sc+3.
            fills = {}
            fills[0] = [lambda: v_transposes(ps_f, "fil", 0, 0)]
            fills[1] = [lambda: v_transposes(ps_f, "fil", 1, 0)]
            fills[2] = [lambda: qkv_chain(ps_f, "fil", 1, 3)]
            fills[3] = [lambda: qkv_chain(ps_f, "fil", 1, 4)]
            fills[4] = [lambda: v_transposes(ps_f, "fil", 0, 1)]
            fills[5] = [lambda: v_transposes(ps_f, "fil", 1, 1)]
            fills[6] = [lambda: v_transposes(ps_f, "fil", 0, 2)]
            fills[7] = [lambda: v_transposes(ps_f, "fil", 1, 2)]
            for n, m in enumerate(range(NM)):
                fills.setdefault(8 + 2 * n, []).append(
                    lambda m=m: qkv_chain(ps_f, "fil", 2, m))
            fills[17] = [lambda: qkv_chain(ps_f, "fil", 3, 0)]
            fills[18] = [lambda: qkv_chain(ps_f, "fil", 3, 2)]
            fills[19] = [lambda: v_transposes(ps_f, "fil", 2, 0)]
            fills[20] = [lambda: qkv_chain(ps_f, "fil", 3, 4)]
            fills[21] = [lambda: v_transposes(ps_f, "fil", 2, 1)]
            fills[22] = [lambda: qkv_chain(ps_f, "fil", 3, 3)]
            fills[23] = [lambda: v_transposes(ps_f, "fil", 2, 2)]
            fills[24] = [lambda: v_transposes(ps_f, "fil", 3, 0)]
            fills[25] = [lambda: v_transposes(ps_f, "fil", 3, 1)]
            fills[26] = [lambda: v_transposes(ps_f, "fil", 3, 2)]
            fills[28] = [lambda: qkv_chain(ps_f, "fil", 3, 1)]
            for n, qt in enumerate(range(0, 4)):        # q-chunk 0
                fills.setdefault(30 + 3 * n, []).append(
                    lambda qt=qt: d_proj(qt))
            for n, qt in enumerate(range(4, 8)):        # q-chunk 1
                fills.setdefault(42 + 3 * n, []).append(
                    lambda qt=qt: d_proj(qt))
            for n, qt in enumerate(range(8, 12)):       # q-chunk 2
                fills.setdefault(70 + n, []).append(
                    lambda qt=qt: d_proj(qt))

            # Flat pipelined stream over all (head, q-half, k-tile) steps,
            # q-half-major so region 1 only needs s-chunks 0,1. Scores run
            # R=2 ahead of PV (matching the 2-slot score pool) so the PE
            # always has queued matmuls while ACT computes the exps.
            sc_list = [(h, 0, t) for h in range(HPC) for t in range(8)] + \
                      [(h, 1, t) for h in range(HPC) for t in range(16)]
            R = 2
            expts = {}
            for k in range(len(sc_list) + R):
                if k < len(sc_list):
                    expts[sc_list[k]] = score_step(*sc_list[k])
                j = k - R
                if j >= 0:
                    h, qp, t = sc_list[j]
                    if t == 0:
                        for qc in (2 * qp, 2 * qp + 1):
                            pouts[(h, qc)] = ps_o.tile([DK + 1, 512], F32,
                                                       tag="pout",
                                                       name=f"po{h}_{qc}")
                    pv_step(h, qp, t, expts.pop(sc_list[j]))
                    if t == 4 * (2 * qp) + 3:
                        divide(h, 2 * qp)
                    if t == 4 * (2 * qp + 1) + 3:
                        divide(h, 2 * qp + 1)
                for fn in fills.pop(k, ()):
                    fn()

            for fns in sorted(fills):
                for fn in fills[fns]:
                    fn()
            for qt in range(12, 16):                    # q-chunk 3 trails
                d_proj(qt)

    nc.compile()
    _NC_CACHE[key] = nc
    return nc


def make_in_maps(X, Wq, Wk, Wv, Wo):
    X = np.ascontiguousarray(np.asarray(X, dtype=np.float32))
    Wq = np.asarray(Wq, dtype=np.float32)
    Wk = np.asarray(Wk, dtype=np.float32)
    Wv = np.asarray(Wv, dtype=np.float32)
    Wo = np.asarray(Wo, dtype=np.float32)

    # causal 0/1 window mask: keep q >= k; rows=k (p), cols=q (f)
    p = np.arange(128)[:, None]
    f = np.arange(128)[None, :]
    mask = (f >= p).astype(np.float16)
    ident = np.eye(128, dtype=np.float32)

    in_maps = []
    for c in range(NCORES):
        b, hg = c // 4, c % 4
        gh = [hg * HPC + l for l in range(HPC)]
        q = [Wq[g * DK:(g + 1) * DK, :] / 8.0 for g in gh]
        k = [Wk[g * DK:(g + 1) * DK, :] for g in gh]
        v = [Wv[g * DK:(g + 1) * DK, :] for g in gh]
        wcat_rows = np.vstack([
            q[0], q[1], k[0], k[1], q[2], v[0], k[2], v[1], v[2],
            np.zeros((DK, D), dtype=np.float32),
        ])                                            # (640, 768)
        wcat = np.ascontiguousarray(
            wcat_rows.T.reshape(NI, 128, NM * 128).astype(ml_dtypes.bfloat16))
        w0, w1, w2 = (Wo[:, g * DK:(g + 1) * DK].T for g in gh)
        wot = np.ascontiguousarray(np.stack([
            np.vstack([w0, w1]),
            np.vstack([w2, np.zeros((DK, D), dtype=np.float32)]),
        ]).astype(np.float16))                                     # (2, 128, 768)
        xt = np.ascontiguousarray(
            X[b].T.reshape(NI, 128, S).astype(ml_dtypes.bfloat16))
        in_maps.append({
            "xt": xt, "wcat": wcat, "wot": wot,
            "mask": mask, "ident": ident,
        })
    return in_maps


def _run(in_maps, trace=False, trace_cores=None):
    nc = build_nc()
    return bass_utils.run_bass_kernel_spmd(
        nc, in_maps, core_ids=list(range(NCORES)),
        trace=trace, trace_cores=trace_cores,
    )


def kernel(X, Wq, Wk, Wv, Wo):
    in_maps = make_in_maps(X, Wq, Wk, Wv, Wo)
    res = _run(in_maps, trace=False)
    out = np.zeros((B, S, D), dtype=np.float32)
    for c in range(NCORES):
        out[c // 4] += res.results[c]["out"]
    return out
